# revision 2
# baseline (speedup 1.0000x reference)
"""Correlation cost-volume kernel for Trainium2 (8 NeuronCores).

out[b,d,h,w] = sum_c left[b,c,h,w] * right[b,c,h,w-shift[d]]
  left/right: [4, 64, 256, 512] f32, shift: arange(96) -> out [4, 96, 256, 512] f32

Strategy:
  - Shard (b, h-half) across 8 cores: per-core left/right [64, 128, 512], no halo
    (shifts are along W only), no collectives.
  - Per (h, w-tile of 128): the cost volume is a 96-wide anti-band of the Gram
    matrix G[i, j] = sum_c L[c, w0+i] * R[c, w0-95+j], computed as one
    TensorEngine matmul [K=64, M=128, N=223] in bf16 (PSUM accumulates f32).
  - Two h rows are packed in partitions 0-63 / 64-127 and run as concurrent
    K=64 matmuls via tile_position row groups.
  - Band extraction: PSUM -> SBUF (bf16 copy) -> DRAM scratch (clean DMA) ->
    diagonal-stride DRAM->DRAM DMA into the output. (SBUF-side diagonal APs
    are miscompiled by the DGEs; DRAM-side diagonal APs execute exactly.)
  - Host: pack/cast inputs to bf16, upcast + transpose + d-flip the output.
"""
import sys

sys.path.insert(0, "/opt/trn_rl_repo")

import numpy as np
import ml_dtypes

import concourse.bass as bass
import concourse.mybir as mybir
import concourse.tile as tile
from concourse.ap import AP
from concourse.bass_utils import run_bass_kernel_spmd
from concourse.vector_clock import ScopedClock

B, C, H, W, D = 4, 64, 256, 512, 96
HC = H // 2          # 128 h rows per core
T = 128              # w-tile size
NT = W // T          # 4 w-tiles
NG = T + D - 1       # 223 gram columns per tile
BLK = 16             # h rows per block
NBLK = HC // BLK     # 8 blocks
PAIR_COLS = (D - 1) + W + W  # 95 pad + 512 R + 512 L = 1119
R_OFF = D - 1        # R data starts at col 95 within a pair's R region
L_OFF = (D - 1) + W  # L data starts at col 607

BF16 = mybir.dt.bfloat16
F32 = mybir.dt.float32


_orig_add_instruction = tile.TileContext._add_instruction


def _patched_add_instruction(self, inst):
    # This walrus build allows at most ONE sync-wait per instruction: peel
    # extra waits onto single-wait NOPs on the same engine, just before it.
    si = inst.sync_info
    if si is not None and len(si.on_wait) > 1:
        waits = list(si.on_wait)
        for w in waits[:-1]:
            nop = mybir.InstNoOp(
                name=self.nc.get_next_instruction_name(),
                text_hint="split_wait",
                bass_nofuse=True,
            )
            nop.engine = inst.engine
            nop.sync_info = mybir.SyncInfo(on_wait=[w], on_update=[])
            _orig_add_instruction(self, nop)
        si.on_wait = waits[-1:]
    _orig_add_instruction(self, inst)


tile.TileContext._add_instruction = _patched_add_instruction


def _patched_drain_and_barrier(self, tick_clock, wait_clock):
    # This walrus build allows only ONE sync-wait on the tail Drain CTRL
    # instruction; split the final-clock waits across single-wait NOPs.
    nc = self.nc
    probe = nc.sync.nop(nofuse=True, hint="drain_waits")
    wait_clock.add_sem_waits(probe.ins, ScopedClock({None: tick_clock.global_clock}))
    waits = list(probe.ins.sync_info.on_wait)
    probe.ins.sync_info.on_wait = waits[:1]
    for w in waits[1:]:
        n = nc.sync.nop(nofuse=True, hint="drain_waits")
        n.ins.sync_info = mybir.SyncInfo(on_wait=[w], on_update=[])
    nc.sync.drain()
    nc.all_engine_barrier()
    assert self.sems is not None
    popped = nc._tile_sem_poison_stack.pop()
    assert popped is self._sem_poison
    nc.clear_and_free_semaphores(list(self.sems.allocated().values()))
    nc.all_engine_barrier()


tile.TileContext._drain_and_barrier = _patched_drain_and_barrier


def build_graph():
    nc = bass.Bass()
    lr_ext = nc.declare_dram_parameter("lrpack", [128, HC // 2, 2 * W], BF16, isOutput=False)
    out_ext = nc.declare_dram_parameter("out", [HC, W, D], BF16, isOutput=True)

    with tile.TileContext(nc) as tc:
        with (
            tc.tile_pool(name="inp", bufs=2) as in_pool,
            tc.tile_pool(name="outsb", bufs=4) as out_pool,
            tc.tile_pool(name="psum", bufs=8, space="PSUM") as psum_pool,
            tc.tile_pool(name="scratch", bufs=2, space="DRAM") as dram_pool,
        ):
            for blk in range(NBLK):
                # ---- load one block: 8 h-pairs -------------------------------
                blk_tile = in_pool.tile([128, (BLK // 2) * PAIR_COLS], BF16)
                # zero the 95-column left-pad of each pair's R region
                pad_ap = AP(
                    tensor=blk_tile.tensor,
                    offset=blk_tile.offset,
                    ap=[[blk_tile.tensor.shape[1], 128], [PAIR_COLS, BLK // 2], [1, R_OFF]],
                )
                nc.vector.memset(pad_ap, 0.0)
                h2_0 = blk * (BLK // 2)
                # L rows -> cols [L_OFF, L_OFF+512) of each pair
                src_l = lr_ext[:, h2_0 : h2_0 + BLK // 2, 0:W]
                dst_l = AP(
                    tensor=blk_tile.tensor,
                    offset=blk_tile.offset + L_OFF,
                    ap=[[blk_tile.tensor.shape[1], 128], [PAIR_COLS, BLK // 2], [1, W]],
                )
                nc.sync.dma_start(dst_l, src_l)
                # R rows -> cols [R_OFF, R_OFF+512)
                src_r = lr_ext[:, h2_0 : h2_0 + BLK // 2, W : 2 * W]
                dst_r = AP(
                    tensor=blk_tile.tensor,
                    offset=blk_tile.offset + R_OFF,
                    ap=[[blk_tile.tensor.shape[1], 128], [PAIR_COLS, BLK // 2], [1, W]],
                )
                nc.sync.dma_start(dst_r, src_r)

                scratch_blk = dram_pool.tile([BLK, 128, NT * NG], BF16)

                # ---- compute: per h row, 4 gram tiles ------------------------
                for j2 in range(BLK // 2):
                    base = j2 * PAIR_COLS
                    for par in range(2):
                        p0 = 64 * par
                        out_sb = out_pool.tile([128, NT * NG], BF16)
                        for t in range(NT):
                            w0 = t * T
                            ps = psum_pool.tile([128, NG], F32)
                            lhsT = blk_tile[p0 : p0 + 64, base + L_OFF + w0 : base + L_OFF + w0 + T]
                            rhs = blk_tile[p0 : p0 + 64, base + w0 : base + w0 + NG]
                            nc.tensor.matmul(
                                ps[:],
                                lhsT=lhsT,
                                rhs=rhs,
                                start=True,
                                stop=True,
                                tile_position=(p0, 0),
                            )
                            dst = out_sb[:, t * NG : (t + 1) * NG]
                            if (2 * j2 + par) % 2 == 0:
                                nc.vector.tensor_copy(dst, ps[:])
                            else:
                                nc.scalar.copy(dst, ps[:])
                        # stage the full h row to DRAM scratch
                        nc.sync.dma_start(scratch_blk[2 * j2 + par], out_sb[:])

                # ---- extract the 96 diagonals: DRAM->DRAM skewed DMAs --------
                for t in range(NT):
                    off_probe = scratch_blk[0, 0, t * NG : t * NG + 1]
                    src = AP(
                        tensor=off_probe.tensor,
                        offset=off_probe.offset,
                        ap=[[128 * NT * NG, BLK], [NT * NG + 1, 128], [1, D]],
                    )
                    dst = AP(
                        tensor=out_ext,
                        offset=(blk * BLK * W + t * T) * D,
                        ap=[[W * D, BLK], [D, 128], [1, D]],
                    )
                    nc.sync.dma_start(dst, src)
    return nc


_CACHED = {}


def _get_graph():
    if "nc" not in _CACHED:
        _CACHED["nc"] = build_graph()
    return _CACHED["nc"]


def _pack_core(left_b, right_b, h0):
    """left_b/right_b: [C, H, W] f32 for one batch -> lrpack [128, 64, 1024] bf16."""
    ls = left_b[:, h0 : h0 + HC, :]
    rs = right_b[:, h0 : h0 + HC, :]
    pack = np.empty((128, HC // 2, 2 * W), dtype=np.float32)
    pack[0:64, :, 0:W] = ls[:, 0::2, :]
    pack[64:128, :, 0:W] = ls[:, 1::2, :]
    pack[0:64, :, W : 2 * W] = rs[:, 0::2, :]
    pack[64:128, :, W : 2 * W] = rs[:, 1::2, :]
    return pack.astype(ml_dtypes.bfloat16)


def _run(inputs, trace=False):
    left = np.asarray(inputs["left"], dtype=np.float32)
    right = np.asarray(inputs["right"], dtype=np.float32)
    shift = np.asarray(inputs["shift"])

    nc = _get_graph()
    in_maps = []
    for core in range(8):
        b, half = core // 2, core % 2
        in_maps.append({"lrpack": _pack_core(left[b], right[b], half * HC)})

    res = run_bass_kernel_spmd(nc, in_maps, core_ids=list(range(8)), trace=trace)

    out = np.empty((B, D, H, W), dtype=np.float32)
    for core in range(8):
        b, half = core // 2, core % 2
        oc = np.asarray(res.results[core]["out"]).astype(np.float32)  # [HC, W, D]
        # out[b, d, h, w] = oc[h, w, 95 - d]
        out[b, :, half * HC : (half + 1) * HC, :] = oc[:, :, ::-1].transpose(2, 0, 1)

    # band covers integer shifts 0..95; remap if shift isn't exactly arange
    s = np.asarray(shift, dtype=np.float64)
    if not np.allclose(s, np.arange(D)):
        si = np.rint(s).astype(np.int64)
        if np.allclose(s, si) and si.min() >= 0 and si.max() < D:
            out = out[:, si, :, :]
        else:
            raise NotImplementedError(f"unsupported shift vector: {s}")
    return out, res


def kernel(**inputs) -> np.ndarray:
    out, _ = _run(inputs, trace=False)
    return out
